# revision 25
# baseline (speedup 1.0000x reference)
"""Trainium2 Bass kernel for nn_CapsuleLayer (dynamic routing), v3.

Problem:  u_hat = einsum('bri,crio->cbro', x, W);  3 routing iterations
          (softmax over R, weighted sum, squash, agreement update).
Shapes:   x [256, 1152, 8] f32, W [10, 1152, 8, 16] f32 ->
          out [10, 256, 1, 1, 16] f32.

Strategy (8 NeuronCores, data-parallel over batch, B_loc = 32/core):
  * all matmul operands fp16 (PE 16-bit streams 1 col/cycle vs fp32 4x);
    accumulation fp32 in PSUM, logits L accumulate fp32 in SBUF;
  * s-pass it0: x-chunk stationaries [128,32] against W moving [128,160]
    giving s[b, co] directly; it1/2: shared W stationaries [128,128]
    covering classes 0-7 with a [128,256] moving of per-class weighted x
    (y8); the (class,class) diagonal blocks of the PSUM result are s;
  * every engine op needs a 32-aligned partition base, so per-class
    16-row data lives in padded 32-row slots (top 16 = data, bottom 16 =
    zeros); the s diagonal is rearranged into that form with PE
    permutation matmuls, per-class norms go through one concatenated
    [128,96] square tile and a single [10,96]-output matmul;
  * agreement: per-wave padded block-diagonal v stationary (3 LDWEIGHTS
    per agreement) streaming resident padded wt in 1024-col chunks;
    U*x on DVE, i-reduction tree on GpSimd in fp16, L accumulated fp32;
  * softmax: ACT exp (fp32, accumulated Z); 1/Z is folded into the PE
    transpose by using diag(1/Z) as the matmul rhs; cwT stored fp16.
"""

import sys
from contextlib import ExitStack

import numpy as np

sys.path.insert(0, "/opt/trn_rl_repo")

import concourse.bacc as bacc
import concourse.bass as bass
import concourse.mybir as mybir
import concourse.tile as tile
from concourse.bass_utils import run_bass_kernel_spmd

F32 = mybir.dt.float32
F16 = mybir.dt.float16
MUL = mybir.AluOpType.mult
ADD = mybir.AluOpType.add
AF = mybir.ActivationFunctionType

B, R, I, C, O = 256, 1152, 8, 10, 16
NC = 8
BL = B // NC          # 32 batch per core
Q = R // 128          # 9 r-blocks of 128
RI = R * I            # 9216
GCH = 1024            # agreement (r,i) chunk
NG = RI // GCH        # 9 chunks
EPS = 1e-7
W_ROWS = (128, 128, 64)   # U-matmul output rows per wave ((k,b) pairs)


def build_nc(debug=False):
    nc = bacc.Bacc("TRN2", target_bir_lowering=False, debug=debug)

    def din(name, shape, dt=F16):
        return nc.declare_dram_parameter(name, shape, dt, isOutput=False)

    xtr_d = din("xtr", [128, Q, I, BL])
    wfr8_d = din("wfr8", [128, Q, I, 128])
    wfr2_d = din("wfr2", [128, Q, I, 32])
    xrep_d = din("xrep", [128, RI])
    wtg0_d = din("wtg0", [128, RI])
    wtg1_d = din("wtg1", [128, RI])
    wtg2_d = din("wtg2", [64, RI])
    id16_d = din("id16", [128, 128])
    id32_d = din("id32", [128, 128], F32)
    p0_d = din("p0", [128, 128], F32)
    p1_d = din("p1", [128, 128], F32)
    p2_d = din("p2", [32, 64], F32)
    e10_d = din("e10", [128, C], F32)
    efa_d = din("efa", [C, 128], F32)
    efb_d = din("efb", [C, 128], F32)
    efc_d = din("efc", [C, 64], F32)
    out_d = nc.declare_dram_parameter("out", [C, O, BL], F32, isOutput=True)

    with tile.TileContext(nc) as tc, ExitStack() as ctx:
        res = ctx.enter_context(tc.tile_pool(name="res", bufs=1))
        cwp = ctx.enter_context(tc.tile_pool(name="cwp", bufs=3))
        y8p = ctx.enter_context(tc.tile_pool(name="y8p", bufs=2))
        ump = ctx.enter_context(tc.tile_pool(name="ump", bufs=2))
        trp = ctx.enter_context(tc.tile_pool(name="trp", bufs=1))
        smp = ctx.enter_context(tc.tile_pool(name="smp", bufs=1))
        psU = ctx.enter_context(
            tc.tile_pool(name="psU", bufs=2, space=bass.MemorySpace.PSUM)
        )
        psP = ctx.enter_context(
            tc.tile_pool(name="psP", bufs=1, space=bass.MemorySpace.PSUM)
        )

        # ---- resident tensors -------------------------------------
        xtr = res.tile([128, Q, I, BL], F16)
        wfr8 = res.tile([128, Q, I, 128], F16)
        wfr2 = res.tile([128, Q, I, 32], F16)
        xrep = res.tile([128, RI], F16)
        wtg0 = res.tile([128, RI], F16)
        wtg1 = res.tile([128, RI], F16)
        wtg2 = res.tile([64, RI], F16)
        id16 = res.tile([128, 128], F16)
        id32 = res.tile([128, 128], F32)
        p0 = res.tile([128, 128], F32)
        p1 = res.tile([128, 128], F32)
        p2 = res.tile([32, 64], F32)
        e10 = res.tile([128, C], F32)
        efa = res.tile([C, 128], F32)
        efb = res.tile([C, 128], F32)
        efc = res.tile([C, 64], F32)
        L = res.tile([128, 3, R], F32)
        cwT = res.tile([128, Q, 3, 128], F16)
        gm2 = res.tile([128, I, R], F16)  # per-wave U*x, (i, r) order
        # padded per-class storage: rows 32k..32k+16 = data, rest zero
        VAB0 = res.tile([128, 128], F16)  # v stationary, classes 0-3
        VAB1 = res.tile([128, 128], F16)  # classes 4-7
        VC = res.tile([64, 64], F16)      # classes 8-9
        s1p0 = res.tile([128, BL], F32)   # padded s, classes 0-3
        s1p1 = res.tile([128, BL], F32)   # classes 4-7
        s2p = res.tile([64, BL], F32)     # classes 8-9
        sqcat = res.tile([128, 96], F32)  # squares, 3 col-blocks
        v0p = res.tile([BL, 256], F16)    # it0 v, col-padded (c, 32)
        v0p2 = res.tile([BL, 64], F16)

        # PSUM (8 banks x 2KB). psU: 2 bufs x [128,1024]f32 = banks 0-3.
        # ps1 bank0 (cols 0:512): psA s-accumulator. bank1 (cols 512:1024):
        # psB/psC accumulators + single-shot outputs. A group's start=True
        # clears has_written bank-wide, so concurrently-accumulating groups
        # (psA vs psB) sit in different banks; single-shot outputs only
        # lose has_written bits, never data.
        ps1 = psP.tile([128, 1024], F32)
        psA = ps1[:, 0:256]
        psB = ps1[0:32, 512:576]
        # psC's two concurrently-accumulating groups must sit in different
        # banks (start=True clears has_written bank-wide)
        psC0 = ps1[0:BL, 256:384]        # bank 0 (idle during it0)
        psC1 = ps1[0:BL, 576:608]        # bank 1
        psP2 = ps1[0:64, 736:800]        # permuted sB
        psn = ps1[0:C, 800:896]          # [10, 96] norms
        psFa = ps1[:, 896:928]           # padded frep / it0 transposed v
        psFb = ps1[:, 928:960]
        psFc = ps1[0:64, 960:992]
        # transpose slots in separate banks: PE writing a bank while another
        # engine reads the same bank is fatal. Also hold permuted sA halves.
        psT6 = psP.tile([128, 256], F32)
        psT7 = psP.tile([128, 256], F32)

        # input DMAs: one descriptor per tensor (descriptor writes are
        # ~680ns each on the sync engine, so fewer is better)
        for t, d in ((xtr, xtr_d), (wfr8, wfr8_d), (wfr2, wfr2_d),
                     (id16, id16_d), (id32, id32_d), (p0, p0_d), (p1, p1_d),
                     (p2, p2_d), (e10, e10_d), (efa, efa_d), (efb, efb_d),
                     (efc, efc_d), (wtg0, wtg0_d), (xrep, xrep_d),
                     (wtg1, wtg1_d), (wtg2, wtg2_d)):
            nc.sync.dma_start(t[:], d[:])
        nc.vector.memset(L[:], 0.0)
        nc.gpsimd.memset(VAB0[:], 0.0)
        nc.gpsimd.memset(VAB1[:], 0.0)
        nc.gpsimd.memset(VC[:], 0.0)
        nc.gpsimd.memset(s1p0[:], 0.0)
        nc.gpsimd.memset(s1p1[:], 0.0)
        nc.gpsimd.memset(s2p[:], 0.0)
        nc.gpsimd.memset(sqcat[:], 0.0)
        nc.gpsimd.memset(v0p[:], 0.0)
        nc.gpsimd.memset(v0p2[:], 0.0)

        # ---------------------------------------------------------------
        def f_chain(snc, p, n):
            """f = (sn/(1+sn)) / sqrt(sn+eps) elementwise on [p, n]."""
            u1 = smp.tile([p, n], F32, tag="u1")
            u2 = smp.tile([p, n], F32, tag="u2")
            u3 = smp.tile([p, n], F32, tag="u3")
            f = smp.tile([p, n], F32, tag="f")
            nc.vector.tensor_scalar_add(u1[:], snc, EPS)
            nc.scalar.sqrt(u2[:], u1[:])
            nc.vector.tensor_scalar_add(u3[:], snc, 1.0)
            nc.vector.tensor_tensor(u1[:], u2[:], u3[:], MUL)
            nc.vector.reciprocal(u2[:], u1[:])
            nc.vector.tensor_tensor(f[:], snc, u2[:], MUL)
            return f

        def fill_v(va, vb, vc):
            """Copy padded v (rows 32k+o) into the block-diag stationaries.
            32-aligned partition bases everywhere; DVE reads PSUM fine."""
            for k in range(4):
                nc.vector.tensor_copy(
                    VAB0[32 * k : 32 * k + 16, 32 * k : 32 * k + 32],
                    va[32 * k : 32 * k + 16, :])
                nc.vector.tensor_copy(
                    VAB1[32 * k : 32 * k + 16, 32 * k : 32 * k + 32],
                    vb[32 * k : 32 * k + 16, :])
            for k in range(2):
                nc.vector.tensor_copy(
                    VC[32 * k : 32 * k + 16, 32 * k : 32 * k + 32],
                    vc[32 * k : 32 * k + 16, :])

        # ---------------------------------------------------------------
        def s_pass0():
            """it0: s[b, co] = sum_{r,i} x W (uniform routing folded later)."""
            for q in range(Q):
                for i in range(I):
                    st = (q == 0 and i == 0)
                    sp = (q == Q - 1 and i == I - 1)
                    nc.tensor.matmul(psC0[:], xtr[:, q, i, :],
                                     wfr8[:, q, i, :], start=st, stop=sp)
                    nc.tensor.matmul(psC1[:], xtr[:, q, i, :],
                                     wfr2[:, q, i, :], start=st, stop=sp)

        def squash0():
            """it0 squash in [b, (c,o)] layout; v transposed into padded form."""
            sC = smp.tile([BL, 160], F32, tag="sC")
            sqC = smp.tile([BL, 160], F32, tag="sqC")
            sn0 = smp.tile([BL, C], F32, tag="sn0")
            nc.scalar.copy(sC[:, 0:128], psC0)
            nc.scalar.copy(sC[:, 128:160], psC1)
            nc.scalar.square(sqC[:], sC[:])
            nc.vector.tensor_reduce(
                sn0[:], sqC.rearrange("b (c o) -> b c o", o=O),
                axis=mybir.AxisListType.X, op=ADD,
            )
            nc.vector.tensor_scalar_mul(sn0[:], sn0[:], 1.0 / (R * R))
            f = f_chain(sn0[:], BL, C)
            nc.vector.tensor_scalar_mul(f[:], f[:], 1.0 / R)
            nc.vector.tensor_tensor(
                v0p.rearrange("b (c oo) -> b c oo", oo=32)[:, :, 0:O],
                sC.rearrange("b (c o) -> b c o", o=O)[:, 0:8, :],
                f[:, 0:8].unsqueeze(-1).broadcast_to([BL, 8, O]),
                MUL,
            )
            nc.vector.tensor_tensor(
                v0p2.rearrange("b (c oo) -> b c oo", oo=32)[:, :, 0:O],
                sC.rearrange("b (c o) -> b c o", o=O)[:, 8:10, :],
                f[:, 8:10].unsqueeze(-1).broadcast_to([BL, 2, O]),
                MUL,
            )
            nc.tensor.matmul(psFa, v0p[:, 0:128], id16[0:BL, 0:BL],
                             start=True, stop=True)
            nc.tensor.matmul(psFb, v0p[:, 128:256], id16[0:BL, 0:BL],
                             start=True, stop=True)
            nc.tensor.matmul(psFc, v0p2[:], id16[0:BL, 0:BL],
                             start=True, stop=True)
            fill_v(psFa, psFb, psFc)

        # ---------------------------------------------------------------
        def build_y(q):
            """y8[i,(c,b)] = cw[c,b,r]*x[b,r,i] classes 0-7; y2 for 8,9.
            Broadcast TTs run ~2ns/elem on both engines; split the q's so
            DVE and GpSimd finish together (gps also carries all y2)."""
            y8 = y8p.tile([128, I, 256], F16, tag="y8")
            y2 = y8p.tile([128, I, 64], F16, tag="y2")
            cw8 = (
                cwT[:, q, 0:2, :]
                .rearrange("p w (k b) -> p (w k) b", b=BL)
                .unsqueeze(1)
                .broadcast_to([128, I, 8, BL])
            )
            xv = xtr[:, q].unsqueeze(2)
            eng = nc.vector if q % 2 == 0 else nc.gpsimd
            eng.tensor_tensor(
                y8.rearrange("p i (c b) -> p i c b", b=BL),
                cw8,
                xv.broadcast_to([128, I, 8, BL]),
                MUL,
            )
            cw2 = (
                cwT[:, q, 2, 0:64]
                .rearrange("p (k b) -> p k b", b=BL)
                .unsqueeze(1)
                .broadcast_to([128, I, 2, BL])
            )
            eng2 = nc.gpsimd if q % 2 == 0 else nc.vector
            eng2.tensor_tensor(
                y2.rearrange("p i (c b) -> p i c b", b=BL),
                cw2,
                xv.broadcast_to([128, I, 2, BL]),
                MUL,
            )
            return y8, y2

        def s_pass(it):
            """it>0: shared W stationaries; diag blocks of psA/psB are s."""
            for q in range(Q):
                y8, y2 = build_y(q)
                for i in range(I):
                    st = (q == 0 and i == 0)
                    sp = (q == Q - 1 and i == I - 1)
                    nc.tensor.matmul(psA, wfr8[:, q, i, :], y8[:, i, :],
                                     start=st, stop=sp)
                    nc.tensor.matmul(psB, wfr2[:, q, i, :], y2[:, i, :],
                                     start=st, stop=sp)

        def squash_co(it):
            """it1/2 squash via PE row-permutation into padded layout."""
            sA = smp.tile([128, 256], F32, tag="sA")
            sB = smp.tile([32, 64], F32, tag="sB")
            nc.scalar.copy(sA[:], psA)
            nc.scalar.copy(sB[:], psB)
            # permute rows 16c+o -> 32k+o so diag blocks sit 32-aligned
            nc.tensor.matmul(psT6[:], p0[:], sA[:], start=True, stop=True)
            nc.tensor.matmul(psT7[:], p1[:], sA[:], start=True, stop=True)
            nc.tensor.matmul(psP2, p2[:], sB[:], start=True, stop=True)
            for k in range(4):
                nc.vector.tensor_copy(
                    s1p0[32 * k : 32 * k + 16, :],
                    psT6[32 * k : 32 * k + 16, 32 * k : 32 * k + 32])
                nc.vector.tensor_copy(
                    s1p1[32 * k : 32 * k + 16, :],
                    psT7[32 * k : 32 * k + 16, 128 + 32 * k : 128 + 32 * k + 32])
            for k in range(2):
                nc.vector.tensor_copy(
                    s2p[32 * k : 32 * k + 16, :],
                    psP2[32 * k : 32 * k + 16, 32 * k : 32 * k + 32])
            nc.scalar.square(sqcat[:, 0:32], s1p0[:])
            nc.scalar.square(sqcat[:, 32:64], s1p1[:])
            nc.scalar.square(sqcat[0:64, 64:96], s2p[:])
            nc.tensor.matmul(psn, e10[:], sqcat[:], start=True, stop=True)
            # f on all 96 cols at once; each class reads its own col-block
            f = f_chain(psn, C, 96)
            nc.tensor.matmul(psFa, efa[:], f[:, 0:32], start=True, stop=True)
            nc.tensor.matmul(psFb, efb[:], f[:, 32:64], start=True, stop=True)
            nc.tensor.matmul(psFc, efc[:], f[:, 64:96], start=True, stop=True)
            if it == 2:
                vpa = smp.tile([128, BL], F32, tag="vpa")
                vpb = smp.tile([128, BL], F32, tag="vpb")
                vpc = smp.tile([64, BL], F32, tag="vpc")
                nc.vector.tensor_tensor(vpa[:], s1p0[:], psFa, MUL)
                nc.vector.tensor_tensor(vpb[:], s1p1[:], psFb, MUL)
                nc.vector.tensor_tensor(vpc[:], s2p[:], psFc, MUL)
                for k in range(4):
                    nc.sync.dma_start(out_d[k],
                                      vpa[32 * k : 32 * k + 16, :])
                    nc.sync.dma_start(out_d[4 + k],
                                      vpb[32 * k : 32 * k + 16, :])
                for k in range(2):
                    nc.sync.dma_start(out_d[8 + k],
                                      vpc[32 * k : 32 * k + 16, :])
            else:
                va = smp.tile([128, BL], F16, tag="va")
                vb = smp.tile([128, BL], F16, tag="vb")
                vc = smp.tile([64, BL], F16, tag="vc")
                nc.vector.tensor_tensor(va[:], s1p0[:], psFa, MUL)
                nc.vector.tensor_tensor(vb[:], s1p1[:], psFb, MUL)
                nc.vector.tensor_tensor(vc[:], s2p[:], psFc, MUL)
                fill_v(va, vb, vc)

        # ---------------------------------------------------------------
        def agreement_softmax():
            """L[p,w,r] += sum_i x*(sum_o v*W); then per-wave softmax to cwT."""
            vsrc = (VAB0[:], VAB1[:], VC[:])
            msrc = (wtg0[:], wtg1[:], wtg2[:])
            for w in range(3):
                rows = W_ROWS[w]
                gmf = gm2.rearrange("p i r -> p (i r)")
                for n in range(NG):
                    off = n * GCH
                    pu = psU.tile([128, GCH], F32, tag="pu")
                    # fp16 moving operand caps at 512 cols; split the chunk
                    for h in range(0, GCH, 512):
                        nc.tensor.matmul(pu[0:rows, h : h + 512], vsrc[w],
                                         msrc[w][:, off + h : off + h + 512],
                                         start=True, stop=True)
                    if n % 2 == 0:
                        # DVE multiplies straight out of PSUM
                        nc.vector.tensor_tensor(
                            gmf[0:rows, off : off + GCH], pu[0:rows, :],
                            xrep[0:rows, off : off + GCH], MUL,
                        )
                    else:
                        # ACT drains to fp16, DVE multiplies at 2x in SBUF
                        um = ump.tile([128, GCH], F16, tag="um")
                        nc.scalar.copy(um[0:rows, :], pu[0:rows, :])
                        nc.vector.tensor_tensor(
                            gmf[0:rows, off : off + GCH], um[0:rows, :],
                            xrep[0:rows, off : off + GCH], MUL,
                        )
                    # HAM keep-warm: PE re-throttles after a fully idle
                    # ~3.4us window; this dummy depends on the chunk's
                    # multiply so it executes mid-gap and keeps PE warm
                    nc.tensor.matmul(psP2, VC[:],
                                     gmf[0:64, off : off + 64],
                                     start=True, stop=True)
                # full-wave i-reduction: contiguous fp16 adds; GpSimd only
                # takes a 1/3 column slice (it measures ~3x slower than DVE)
                l1 = trp.tile([128, 4, R], F16, tag="l1")
                l2 = trp.tile([128, 2, R], F16, tag="l2")
                a = trp.tile([128, R], F16, tag="a")
                RS = 768
                nc.vector.tensor_tensor(
                    l1[0:rows, :, 0:RS], gm2[0:rows, 0:4, 0:RS],
                    gm2[0:rows, 4:8, 0:RS], ADD)
                nc.gpsimd.tensor_tensor(
                    l1[0:rows, :, RS:R], gm2[0:rows, 0:4, RS:R],
                    gm2[0:rows, 4:8, RS:R], ADD)
                nc.vector.tensor_tensor(
                    l2[0:rows], l1[0:rows, 0:2, :], l1[0:rows, 2:4, :], ADD
                )
                nc.gpsimd.tensor_tensor(
                    a[0:rows], l2[0:rows, 0, :], l2[0:rows, 1, :], ADD
                )
                nc.vector.tensor_tensor(
                    L[0:rows, w, :], L[0:rows, w, :], a[0:rows], ADD
                )
                # per-wave softmax + transposes: emitting transposes here
                # lets the next iteration's y8 builds start while later
                # waves are still in their agreement chunks
                cwv = cwp.tile([128, R], F32, tag="cwv")
                Zt = smp.tile([128, 1], F32, tag="Zt")
                Zi = smp.tile([128, 1], F32, tag="Zi")
                nc.scalar.activation(cwv[0:rows, :], L[0:rows, w, :], AF.Exp,
                                     accum_out=Zt[0:rows])
                nc.vector.reciprocal(Zi[0:rows], Zt[0:rows])
                nc.vector.tensor_scalar_mul(cwv[0:rows, :], cwv[0:rows, :],
                                            Zi[0:rows])
                for q in range(Q):
                    pt = (psT6, psT7)[q % 2]
                    nc.tensor.matmul(pt[:, 0:rows],
                                     cwv[0:rows, 128 * q : 128 * (q + 1)],
                                     id32[0:rows, 0:rows], start=True, stop=True)
                    nc.scalar.copy(cwT[:, q, w, 0:rows], pt[:, 0:rows])

        # =========================== flow ==============================
        s_pass0()
        squash0()
        agreement_softmax()
        s_pass(1)
        squash_co(it=1)
        agreement_softmax()
        s_pass(2)
        squash_co(it=2)

    nc.compile()
    return nc


# =================== host-side prep / entry point =====================

def _prep_shared(W):
    """Per-problem constant tensors (replicated on every core)."""
    W = np.ascontiguousarray(W, np.float32)
    wfr8 = np.ascontiguousarray(
        W[:8].reshape(8, Q, 128, I, O).transpose(2, 1, 3, 0, 4).reshape(128, Q, I, 128)
    ).astype(np.float16)
    wfr2 = np.ascontiguousarray(
        W[8:].reshape(2, Q, 128, I, O).transpose(2, 1, 3, 0, 4).reshape(128, Q, I, 32)
    ).astype(np.float16)
    # padded 32-row class slots
    wtg0 = np.zeros((128, RI), np.float16)
    wtg1 = np.zeros((128, RI), np.float16)
    wtg2 = np.zeros((64, RI), np.float16)
    # column order (i, r): col = i*R + r  (makes the i-reduce contiguous)
    for k in range(4):
        wtg0[32 * k : 32 * k + 16] = W[k].transpose(2, 1, 0).reshape(O, RI)
        wtg1[32 * k : 32 * k + 16] = W[4 + k].transpose(2, 1, 0).reshape(O, RI)
    for k in range(2):
        wtg2[32 * k : 32 * k + 16] = W[8 + k].transpose(2, 1, 0).reshape(O, RI)
    id16 = np.eye(128, dtype=np.float16)
    id32 = np.eye(128, dtype=np.float32)
    # row permutations compact [16c+o] -> padded [32k+o]
    p0 = np.zeros((128, 128), np.float32)
    p1 = np.zeros((128, 128), np.float32)
    p2 = np.zeros((32, 64), np.float32)
    for o in range(O):
        for k in range(4):
            p0[16 * k + o, 32 * k + o] = 1.0
            p1[16 * (4 + k) + o, 32 * k + o] = 1.0
        for k in range(2):
            p2[16 * k + o, 32 * k + o] = 1.0
    # per-class norm reduce: psn[c, :] = sum_o sq[32k+o, :]
    e10 = np.zeros((128, C), np.float32)
    for o in range(O):
        for k in range(4):
            e10[32 * k + o, k] = 1.0
            e10[32 * k + o, 4 + k] = 1.0
        for k in range(2):
            e10[32 * k + o, 8 + k] = 1.0
    # padded frep: frep[32k+oo] = f[class(k)] for all oo
    efa = np.zeros((C, 128), np.float32)
    efb = np.zeros((C, 128), np.float32)
    efc = np.zeros((C, 64), np.float32)
    for k in range(4):
        efa[k, 32 * k : 32 * k + 32] = 1.0
        efb[4 + k, 32 * k : 32 * k + 32] = 1.0
    for k in range(2):
        efc[8 + k, 32 * k : 32 * k + 32] = 1.0
    return {
        "wfr8": wfr8, "wfr2": wfr2, "wtg0": wtg0, "wtg1": wtg1, "wtg2": wtg2,
        "id16": id16, "id32": id32, "p0": p0, "p1": p1, "p2": p2,
        "e10": e10, "efa": efa, "efb": efb, "efc": efc,
    }


def _prep_core(x_shard):
    """Per-core tensors for one 32-batch shard."""
    xs = np.ascontiguousarray(x_shard, np.float32)       # [32, 1152, 8]
    xtr = np.ascontiguousarray(
        xs.reshape(BL, Q, 128, I).transpose(2, 1, 3, 0)
    ).astype(np.float16)                                  # [128, Q, I, 32]
    flat = xs.transpose(0, 2, 1).reshape(BL, RI)          # (i, r) order
    xrep = np.ascontiguousarray(
        flat[np.arange(128) % BL].astype(np.float16)
    )                                                     # [128, RI]
    return {"xtr": xtr, "xrep": xrep}


_NC_CACHE = {}


def kernel(x, W):
    x = np.asarray(x, np.float32)
    W = np.asarray(W, np.float32)
    if "nc" not in _NC_CACHE:
        _NC_CACHE["nc"] = build_nc()
    nc = _NC_CACHE["nc"]

    shared = _prep_shared(W)
    in_maps = []
    for m in range(NC):
        per = _prep_core(x[m * BL : (m + 1) * BL])
        in_maps.append({**shared, **per})

    res = run_bass_kernel_spmd(nc, in_maps, list(range(NC)))
    out = np.empty((C, B, 1, 1, O), np.float32)
    for m in range(NC):
        o = res.results[m]["out"]                         # [C, O, BL]
        out[:, m * BL : (m + 1) * BL, 0, 0, :] = np.asarray(o).transpose(0, 2, 1)
    return out


if __name__ == "__main__":
    d = np.load("/root/problem/ref_data.npz")
    got = kernel(d["x"], d["W"])
    exp = d["expected"]
    err = np.abs(got - exp).max() / np.abs(exp).max()
    print("Relative error:", err)


# revision 26
# speedup vs baseline: 1.0246x; 1.0246x over previous
"""Trainium2 Bass kernel for nn_CapsuleLayer (dynamic routing), v3.

Problem:  u_hat = einsum('bri,crio->cbro', x, W);  3 routing iterations
          (softmax over R, weighted sum, squash, agreement update).
Shapes:   x [256, 1152, 8] f32, W [10, 1152, 8, 16] f32 ->
          out [10, 256, 1, 1, 16] f32.

Strategy (8 NeuronCores, data-parallel over batch, B_loc = 32/core):
  * all matmul operands fp16 (PE 16-bit streams 1 col/cycle vs fp32 4x);
    accumulation fp32 in PSUM, logits L accumulate fp32 in SBUF;
  * s-pass it0: x-chunk stationaries [128,32] against W moving [128,160]
    giving s[b, co] directly; it1/2: shared W stationaries [128,128]
    covering classes 0-7 with a [128,256] moving of per-class weighted x
    (y8); the (class,class) diagonal blocks of the PSUM result are s;
  * every engine op needs a 32-aligned partition base, so per-class
    16-row data lives in padded 32-row slots (top 16 = data, bottom 16 =
    zeros); the s diagonal is rearranged into that form with PE
    permutation matmuls, per-class norms go through one concatenated
    [128,96] square tile and a single [10,96]-output matmul;
  * agreement: per-wave padded block-diagonal v stationary (3 LDWEIGHTS
    per agreement) streaming resident padded wt in 1024-col chunks;
    U*x on DVE, i-reduction tree on GpSimd in fp16, L accumulated fp32;
  * softmax: ACT exp (fp32, accumulated Z); 1/Z is folded into the PE
    transpose by using diag(1/Z) as the matmul rhs; cwT stored fp16.
"""

import sys
from contextlib import ExitStack

import numpy as np

sys.path.insert(0, "/opt/trn_rl_repo")

import concourse.bacc as bacc
import concourse.bass as bass
import concourse.mybir as mybir
import concourse.tile as tile
from concourse.bass_utils import run_bass_kernel_spmd

F32 = mybir.dt.float32
F16 = mybir.dt.float16
MUL = mybir.AluOpType.mult
ADD = mybir.AluOpType.add
AF = mybir.ActivationFunctionType

B, R, I, C, O = 256, 1152, 8, 10, 16
NC = 8
BL = B // NC          # 32 batch per core
Q = R // 128          # 9 r-blocks of 128
RI = R * I            # 9216
GCH = 1024            # agreement (r,i) chunk
NG = RI // GCH        # 9 chunks
EPS = 1e-7
W_ROWS = (128, 128, 64)   # U-matmul output rows per wave ((k,b) pairs)


def build_nc(debug=False):
    nc = bacc.Bacc("TRN2", target_bir_lowering=False, debug=debug)

    def din(name, shape, dt=F16):
        return nc.declare_dram_parameter(name, shape, dt, isOutput=False)

    xtr_d = din("xtr", [128, Q, I, BL])
    wfr8_d = din("wfr8", [128, Q, I, 128])
    wfr2_d = din("wfr2", [128, Q, I, 32])
    xrep_d = din("xrep", [128, RI])
    wtg0_d = din("wtg0", [128, RI])
    wtg1_d = din("wtg1", [128, RI])
    wtg2_d = din("wtg2", [64, RI])
    id16_d = din("id16", [128, 128])
    id32_d = din("id32", [128, 128], F32)
    p0_d = din("p0", [128, 128], F32)
    p1_d = din("p1", [128, 128], F32)
    p2_d = din("p2", [32, 64], F32)
    e10_d = din("e10", [128, C], F32)
    efa_d = din("efa", [C, 128], F32)
    efb_d = din("efb", [C, 128], F32)
    efc_d = din("efc", [C, 64], F32)
    out_d = nc.declare_dram_parameter("out", [C, O, BL], F32, isOutput=True)

    with tile.TileContext(nc) as tc, ExitStack() as ctx:
        res = ctx.enter_context(tc.tile_pool(name="res", bufs=1))
        cwp = ctx.enter_context(tc.tile_pool(name="cwp", bufs=3))
        y8p = ctx.enter_context(tc.tile_pool(name="y8p", bufs=2))
        ump = ctx.enter_context(tc.tile_pool(name="ump", bufs=3))
        trp = ctx.enter_context(tc.tile_pool(name="trp", bufs=1))
        smp = ctx.enter_context(tc.tile_pool(name="smp", bufs=1))
        psU = ctx.enter_context(
            tc.tile_pool(name="psU", bufs=2, space=bass.MemorySpace.PSUM)
        )
        psP = ctx.enter_context(
            tc.tile_pool(name="psP", bufs=1, space=bass.MemorySpace.PSUM)
        )

        # ---- resident tensors -------------------------------------
        xtr = res.tile([128, Q, I, BL], F16)
        wfr8 = res.tile([128, Q, I, 128], F16)
        wfr2 = res.tile([128, Q, I, 32], F16)
        xrep = res.tile([128, RI], F16)
        wtg0 = res.tile([128, RI], F16)
        wtg1 = res.tile([128, RI], F16)
        wtg2 = res.tile([64, RI], F16)
        id16 = res.tile([128, 128], F16)
        id32 = res.tile([128, 128], F32)
        p0 = res.tile([128, 128], F32)
        p1 = res.tile([128, 128], F32)
        p2 = res.tile([32, 64], F32)
        e10 = res.tile([128, C], F32)
        efa = res.tile([C, 128], F32)
        efb = res.tile([C, 128], F32)
        efc = res.tile([C, 64], F32)
        L = res.tile([128, 3, R], F32)
        cwT = res.tile([128, Q, 3, 128], F16)
        gm2 = res.tile([128, I, R], F16)  # per-wave U*x, (i, r) order
        # padded per-class storage: rows 32k..32k+16 = data, rest zero
        VAB0 = res.tile([128, 128], F16)  # v stationary, classes 0-3
        VAB1 = res.tile([128, 128], F16)  # classes 4-7
        VC = res.tile([64, 64], F16)      # classes 8-9
        s1p0 = res.tile([128, BL], F32)   # padded s, classes 0-3
        s1p1 = res.tile([128, BL], F32)   # classes 4-7
        s2p = res.tile([64, BL], F32)     # classes 8-9
        sqcat = res.tile([128, 96], F32)  # squares, 3 col-blocks
        v0p = res.tile([BL, 256], F16)    # it0 v, col-padded (c, 32)
        v0p2 = res.tile([BL, 64], F16)

        # PSUM (8 banks x 2KB). psU: 2 bufs x [128,1024]f32 = banks 0-3.
        # ps1 bank0 (cols 0:512): psA s-accumulator. bank1 (cols 512:1024):
        # psB/psC accumulators + single-shot outputs. A group's start=True
        # clears has_written bank-wide, so concurrently-accumulating groups
        # (psA vs psB) sit in different banks; single-shot outputs only
        # lose has_written bits, never data.
        ps1 = psP.tile([128, 1024], F32)
        psA = ps1[:, 0:256]
        psB = ps1[0:32, 512:576]
        # psC's two concurrently-accumulating groups must sit in different
        # banks (start=True clears has_written bank-wide)
        psC0 = ps1[0:BL, 256:384]        # bank 0 (idle during it0)
        psC1 = ps1[0:BL, 576:608]        # bank 1
        psP2 = ps1[0:64, 736:800]        # permuted sB
        psn = ps1[0:C, 800:896]          # [10, 96] norms
        psFa = ps1[:, 896:928]           # padded frep / it0 transposed v
        psFb = ps1[:, 928:960]
        psFc = ps1[0:64, 960:992]
        # transpose slots in separate banks: PE writing a bank while another
        # engine reads the same bank is fatal. Also hold permuted sA halves.
        psT6 = psP.tile([128, 256], F32)
        psT7 = psP.tile([128, 256], F32)

        # input DMAs: one descriptor per tensor (descriptor writes are
        # ~680ns each on the sync engine, so fewer is better)
        for t, d in ((xtr, xtr_d), (wfr8, wfr8_d), (wfr2, wfr2_d),
                     (id16, id16_d), (id32, id32_d), (p0, p0_d), (p1, p1_d),
                     (p2, p2_d), (e10, e10_d), (efa, efa_d), (efb, efb_d),
                     (efc, efc_d), (wtg0, wtg0_d), (xrep, xrep_d),
                     (wtg1, wtg1_d), (wtg2, wtg2_d)):
            nc.sync.dma_start(t[:], d[:])
        nc.vector.memset(L[:], 0.0)
        nc.gpsimd.memset(VAB0[:], 0.0)
        nc.gpsimd.memset(VAB1[:], 0.0)
        nc.gpsimd.memset(VC[:], 0.0)
        nc.gpsimd.memset(s1p0[:], 0.0)
        nc.gpsimd.memset(s1p1[:], 0.0)
        nc.gpsimd.memset(s2p[:], 0.0)
        nc.gpsimd.memset(sqcat[:], 0.0)
        nc.gpsimd.memset(v0p[:], 0.0)
        nc.gpsimd.memset(v0p2[:], 0.0)

        # ---------------------------------------------------------------
        def f_chain(snc, p, n):
            """f = (sn/(1+sn)) / sqrt(sn+eps) elementwise on [p, n]."""
            u1 = smp.tile([p, n], F32, tag="u1")
            u2 = smp.tile([p, n], F32, tag="u2")
            u3 = smp.tile([p, n], F32, tag="u3")
            f = smp.tile([p, n], F32, tag="f")
            nc.vector.tensor_scalar_add(u1[:], snc, EPS)
            nc.scalar.sqrt(u2[:], u1[:])
            nc.vector.tensor_scalar_add(u3[:], snc, 1.0)
            nc.vector.tensor_tensor(u1[:], u2[:], u3[:], MUL)
            nc.vector.reciprocal(u2[:], u1[:])
            nc.vector.tensor_tensor(f[:], snc, u2[:], MUL)
            return f

        def fill_v(va, vb, vc):
            """Copy padded v (rows 32k+o) into the block-diag stationaries.
            32-aligned partition bases everywhere; DVE reads PSUM fine."""
            for k in range(4):
                nc.vector.tensor_copy(
                    VAB0[32 * k : 32 * k + 16, 32 * k : 32 * k + 32],
                    va[32 * k : 32 * k + 16, :])
                nc.vector.tensor_copy(
                    VAB1[32 * k : 32 * k + 16, 32 * k : 32 * k + 32],
                    vb[32 * k : 32 * k + 16, :])
            for k in range(2):
                nc.vector.tensor_copy(
                    VC[32 * k : 32 * k + 16, 32 * k : 32 * k + 32],
                    vc[32 * k : 32 * k + 16, :])

        # ---------------------------------------------------------------
        def s_pass0():
            """it0: s[b, co] = sum_{r,i} x W (uniform routing folded later)."""
            for q in range(Q):
                for i in range(I):
                    st = (q == 0 and i == 0)
                    sp = (q == Q - 1 and i == I - 1)
                    nc.tensor.matmul(psC0[:], xtr[:, q, i, :],
                                     wfr8[:, q, i, :], start=st, stop=sp)
                    nc.tensor.matmul(psC1[:], xtr[:, q, i, :],
                                     wfr2[:, q, i, :], start=st, stop=sp)

        def squash0():
            """it0 squash in [b, (c,o)] layout; v transposed into padded form."""
            sC = smp.tile([BL, 160], F32, tag="sC")
            sqC = smp.tile([BL, 160], F32, tag="sqC")
            sn0 = smp.tile([BL, C], F32, tag="sn0")
            nc.scalar.copy(sC[:, 0:128], psC0)
            nc.scalar.copy(sC[:, 128:160], psC1)
            nc.scalar.square(sqC[:], sC[:])
            nc.vector.tensor_reduce(
                sn0[:], sqC.rearrange("b (c o) -> b c o", o=O),
                axis=mybir.AxisListType.X, op=ADD,
            )
            nc.vector.tensor_scalar_mul(sn0[:], sn0[:], 1.0 / (R * R))
            f = f_chain(sn0[:], BL, C)
            nc.vector.tensor_scalar_mul(f[:], f[:], 1.0 / R)
            nc.vector.tensor_tensor(
                v0p.rearrange("b (c oo) -> b c oo", oo=32)[:, :, 0:O],
                sC.rearrange("b (c o) -> b c o", o=O)[:, 0:8, :],
                f[:, 0:8].unsqueeze(-1).broadcast_to([BL, 8, O]),
                MUL,
            )
            nc.vector.tensor_tensor(
                v0p2.rearrange("b (c oo) -> b c oo", oo=32)[:, :, 0:O],
                sC.rearrange("b (c o) -> b c o", o=O)[:, 8:10, :],
                f[:, 8:10].unsqueeze(-1).broadcast_to([BL, 2, O]),
                MUL,
            )
            nc.tensor.matmul(psFa, v0p[:, 0:128], id16[0:BL, 0:BL],
                             start=True, stop=True)
            nc.tensor.matmul(psFb, v0p[:, 128:256], id16[0:BL, 0:BL],
                             start=True, stop=True)
            nc.tensor.matmul(psFc, v0p2[:], id16[0:BL, 0:BL],
                             start=True, stop=True)
            fill_v(psFa, psFb, psFc)

        # ---------------------------------------------------------------
        def build_y(q):
            """y8[i,(c,b)] = cw[c,b,r]*x[b,r,i] classes 0-7; y2 for 8,9.
            Broadcast TTs run ~2ns/elem on both engines; split the q's so
            DVE and GpSimd finish together (gps also carries all y2)."""
            y8 = y8p.tile([128, I, 256], F16, tag="y8")
            y2 = y8p.tile([128, I, 64], F16, tag="y2")
            cw8 = (
                cwT[:, q, 0:2, :]
                .rearrange("p w (k b) -> p (w k) b", b=BL)
                .unsqueeze(1)
                .broadcast_to([128, I, 8, BL])
            )
            xv = xtr[:, q].unsqueeze(2)
            eng = nc.vector if q % 2 == 0 else nc.gpsimd
            eng.tensor_tensor(
                y8.rearrange("p i (c b) -> p i c b", b=BL),
                cw8,
                xv.broadcast_to([128, I, 8, BL]),
                MUL,
            )
            cw2 = (
                cwT[:, q, 2, 0:64]
                .rearrange("p (k b) -> p k b", b=BL)
                .unsqueeze(1)
                .broadcast_to([128, I, 2, BL])
            )
            eng2 = nc.gpsimd if q % 2 == 0 else nc.vector
            eng2.tensor_tensor(
                y2.rearrange("p i (c b) -> p i c b", b=BL),
                cw2,
                xv.broadcast_to([128, I, 2, BL]),
                MUL,
            )
            return y8, y2

        def s_pass(it):
            """it>0: shared W stationaries; diag blocks of psA/psB are s."""
            for q in range(Q):
                y8, y2 = build_y(q)
                for i in range(I):
                    st = (q == 0 and i == 0)
                    sp = (q == Q - 1 and i == I - 1)
                    nc.tensor.matmul(psA, wfr8[:, q, i, :], y8[:, i, :],
                                     start=st, stop=sp)
                    nc.tensor.matmul(psB, wfr2[:, q, i, :], y2[:, i, :],
                                     start=st, stop=sp)

        def squash_co(it):
            """it1/2 squash via PE row-permutation into padded layout."""
            sA = smp.tile([128, 256], F32, tag="sA")
            sB = smp.tile([32, 64], F32, tag="sB")
            nc.scalar.copy(sA[:], psA)
            nc.scalar.copy(sB[:], psB)
            # permute rows 16c+o -> 32k+o so diag blocks sit 32-aligned
            nc.tensor.matmul(psT6[:], p0[:], sA[:], start=True, stop=True)
            nc.tensor.matmul(psT7[:], p1[:], sA[:], start=True, stop=True)
            nc.tensor.matmul(psP2, p2[:], sB[:], start=True, stop=True)
            for k in range(4):
                nc.vector.tensor_copy(
                    s1p0[32 * k : 32 * k + 16, :],
                    psT6[32 * k : 32 * k + 16, 32 * k : 32 * k + 32])
                nc.vector.tensor_copy(
                    s1p1[32 * k : 32 * k + 16, :],
                    psT7[32 * k : 32 * k + 16, 128 + 32 * k : 128 + 32 * k + 32])
            for k in range(2):
                nc.vector.tensor_copy(
                    s2p[32 * k : 32 * k + 16, :],
                    psP2[32 * k : 32 * k + 16, 32 * k : 32 * k + 32])
            nc.scalar.square(sqcat[:, 0:32], s1p0[:])
            nc.scalar.square(sqcat[:, 32:64], s1p1[:])
            nc.scalar.square(sqcat[0:64, 64:96], s2p[:])
            nc.tensor.matmul(psn, e10[:], sqcat[:], start=True, stop=True)
            # f on all 96 cols at once; each class reads its own col-block
            f = f_chain(psn, C, 96)
            nc.tensor.matmul(psFa, efa[:], f[:, 0:32], start=True, stop=True)
            nc.tensor.matmul(psFb, efb[:], f[:, 32:64], start=True, stop=True)
            nc.tensor.matmul(psFc, efc[:], f[:, 64:96], start=True, stop=True)
            if it == 2:
                vpa = smp.tile([128, BL], F32, tag="vpa")
                vpb = smp.tile([128, BL], F32, tag="vpb")
                vpc = smp.tile([64, BL], F32, tag="vpc")
                nc.vector.tensor_tensor(vpa[:], s1p0[:], psFa, MUL)
                nc.vector.tensor_tensor(vpb[:], s1p1[:], psFb, MUL)
                nc.vector.tensor_tensor(vpc[:], s2p[:], psFc, MUL)
                for k in range(4):
                    nc.sync.dma_start(out_d[k],
                                      vpa[32 * k : 32 * k + 16, :])
                    nc.sync.dma_start(out_d[4 + k],
                                      vpb[32 * k : 32 * k + 16, :])
                for k in range(2):
                    nc.sync.dma_start(out_d[8 + k],
                                      vpc[32 * k : 32 * k + 16, :])
            else:
                va = smp.tile([128, BL], F16, tag="va")
                vb = smp.tile([128, BL], F16, tag="vb")
                vc = smp.tile([64, BL], F16, tag="vc")
                nc.vector.tensor_tensor(va[:], s1p0[:], psFa, MUL)
                nc.vector.tensor_tensor(vb[:], s1p1[:], psFb, MUL)
                nc.vector.tensor_tensor(vc[:], s2p[:], psFc, MUL)
                fill_v(va, vb, vc)

        # ---------------------------------------------------------------
        def agreement_softmax():
            """L[p,w,r] += sum_i x*(sum_o v*W); then per-wave softmax to cwT."""
            vsrc = (VAB0[:], VAB1[:], VC[:])
            msrc = (wtg0[:], wtg1[:], wtg2[:])
            for w in range(3):
                rows = W_ROWS[w]
                gmf = gm2.rearrange("p i r -> p (i r)")
                for n in range(NG):
                    off = n * GCH
                    pu = psU.tile([128, GCH], F32, tag="pu")
                    # fp16 moving operand caps at 512 cols; split the chunk
                    for h in range(0, GCH, 512):
                        nc.tensor.matmul(pu[0:rows, h : h + 512], vsrc[w],
                                         msrc[w][:, off + h : off + h + 512],
                                         start=True, stop=True)
                    if n % 3 == 0:
                        # DVE multiplies straight out of PSUM
                        nc.vector.tensor_tensor(
                            gmf[0:rows, off : off + GCH], pu[0:rows, :],
                            xrep[0:rows, off : off + GCH], MUL,
                        )
                    else:
                        # ACT drains to fp16, DVE multiplies at 2x in SBUF
                        um = ump.tile([128, GCH], F16, tag="um")
                        nc.scalar.copy(um[0:rows, :], pu[0:rows, :])
                        nc.vector.tensor_tensor(
                            gmf[0:rows, off : off + GCH], um[0:rows, :],
                            xrep[0:rows, off : off + GCH], MUL,
                        )
                    # HAM keep-warm: PE re-throttles after a fully idle
                    # ~3.4us window; this dummy depends on the chunk's
                    # multiply so it executes mid-gap and keeps PE warm
                    nc.tensor.matmul(psP2, VC[:],
                                     gmf[0:64, off : off + 64],
                                     start=True, stop=True)
                # full-wave i-reduction: contiguous fp16 adds; GpSimd only
                # takes a 1/3 column slice (it measures ~3x slower than DVE)
                l1 = trp.tile([128, 4, R], F16, tag="l1")
                l2 = trp.tile([128, 2, R], F16, tag="l2")
                a = trp.tile([128, R], F16, tag="a")
                RS = 768
                nc.vector.tensor_tensor(
                    l1[0:rows, :, 0:RS], gm2[0:rows, 0:4, 0:RS],
                    gm2[0:rows, 4:8, 0:RS], ADD)
                nc.gpsimd.tensor_tensor(
                    l1[0:rows, :, RS:R], gm2[0:rows, 0:4, RS:R],
                    gm2[0:rows, 4:8, RS:R], ADD)
                nc.vector.tensor_tensor(
                    l2[0:rows], l1[0:rows, 0:2, :], l1[0:rows, 2:4, :], ADD
                )
                nc.gpsimd.tensor_tensor(
                    a[0:rows], l2[0:rows, 0, :], l2[0:rows, 1, :], ADD
                )
                nc.vector.tensor_tensor(
                    L[0:rows, w, :], L[0:rows, w, :], a[0:rows], ADD
                )
                # per-wave softmax + transposes: emitting transposes here
                # lets the next iteration's y8 builds start while later
                # waves are still in their agreement chunks
                cwv = cwp.tile([128, R], F32, tag="cwv")
                Zt = smp.tile([128, 1], F32, tag="Zt")
                Zi = smp.tile([128, 1], F32, tag="Zi")
                nc.scalar.activation(cwv[0:rows, :], L[0:rows, w, :], AF.Exp,
                                     accum_out=Zt[0:rows])
                nc.vector.reciprocal(Zi[0:rows], Zt[0:rows])
                nc.vector.tensor_scalar_mul(cwv[0:rows, :], cwv[0:rows, :],
                                            Zi[0:rows])
                for q in range(Q):
                    pt = (psT6, psT7)[q % 2]
                    nc.tensor.matmul(pt[:, 0:rows],
                                     cwv[0:rows, 128 * q : 128 * (q + 1)],
                                     id32[0:rows, 0:rows], start=True, stop=True)
                    nc.scalar.copy(cwT[:, q, w, 0:rows], pt[:, 0:rows])

        # =========================== flow ==============================
        s_pass0()
        squash0()
        agreement_softmax()
        s_pass(1)
        squash_co(it=1)
        agreement_softmax()
        s_pass(2)
        squash_co(it=2)

    nc.compile()
    return nc


# =================== host-side prep / entry point =====================

def _prep_shared(W):
    """Per-problem constant tensors (replicated on every core)."""
    W = np.ascontiguousarray(W, np.float32)
    wfr8 = np.ascontiguousarray(
        W[:8].reshape(8, Q, 128, I, O).transpose(2, 1, 3, 0, 4).reshape(128, Q, I, 128)
    ).astype(np.float16)
    wfr2 = np.ascontiguousarray(
        W[8:].reshape(2, Q, 128, I, O).transpose(2, 1, 3, 0, 4).reshape(128, Q, I, 32)
    ).astype(np.float16)
    # padded 32-row class slots
    wtg0 = np.zeros((128, RI), np.float16)
    wtg1 = np.zeros((128, RI), np.float16)
    wtg2 = np.zeros((64, RI), np.float16)
    # column order (i, r): col = i*R + r  (makes the i-reduce contiguous)
    for k in range(4):
        wtg0[32 * k : 32 * k + 16] = W[k].transpose(2, 1, 0).reshape(O, RI)
        wtg1[32 * k : 32 * k + 16] = W[4 + k].transpose(2, 1, 0).reshape(O, RI)
    for k in range(2):
        wtg2[32 * k : 32 * k + 16] = W[8 + k].transpose(2, 1, 0).reshape(O, RI)
    id16 = np.eye(128, dtype=np.float16)
    id32 = np.eye(128, dtype=np.float32)
    # row permutations compact [16c+o] -> padded [32k+o]
    p0 = np.zeros((128, 128), np.float32)
    p1 = np.zeros((128, 128), np.float32)
    p2 = np.zeros((32, 64), np.float32)
    for o in range(O):
        for k in range(4):
            p0[16 * k + o, 32 * k + o] = 1.0
            p1[16 * (4 + k) + o, 32 * k + o] = 1.0
        for k in range(2):
            p2[16 * k + o, 32 * k + o] = 1.0
    # per-class norm reduce: psn[c, :] = sum_o sq[32k+o, :]
    e10 = np.zeros((128, C), np.float32)
    for o in range(O):
        for k in range(4):
            e10[32 * k + o, k] = 1.0
            e10[32 * k + o, 4 + k] = 1.0
        for k in range(2):
            e10[32 * k + o, 8 + k] = 1.0
    # padded frep: frep[32k+oo] = f[class(k)] for all oo
    efa = np.zeros((C, 128), np.float32)
    efb = np.zeros((C, 128), np.float32)
    efc = np.zeros((C, 64), np.float32)
    for k in range(4):
        efa[k, 32 * k : 32 * k + 32] = 1.0
        efb[4 + k, 32 * k : 32 * k + 32] = 1.0
    for k in range(2):
        efc[8 + k, 32 * k : 32 * k + 32] = 1.0
    return {
        "wfr8": wfr8, "wfr2": wfr2, "wtg0": wtg0, "wtg1": wtg1, "wtg2": wtg2,
        "id16": id16, "id32": id32, "p0": p0, "p1": p1, "p2": p2,
        "e10": e10, "efa": efa, "efb": efb, "efc": efc,
    }


def _prep_core(x_shard):
    """Per-core tensors for one 32-batch shard."""
    xs = np.ascontiguousarray(x_shard, np.float32)       # [32, 1152, 8]
    xtr = np.ascontiguousarray(
        xs.reshape(BL, Q, 128, I).transpose(2, 1, 3, 0)
    ).astype(np.float16)                                  # [128, Q, I, 32]
    flat = xs.transpose(0, 2, 1).reshape(BL, RI)          # (i, r) order
    xrep = np.ascontiguousarray(
        flat[np.arange(128) % BL].astype(np.float16)
    )                                                     # [128, RI]
    return {"xtr": xtr, "xrep": xrep}


_NC_CACHE = {}


def kernel(x, W):
    x = np.asarray(x, np.float32)
    W = np.asarray(W, np.float32)
    if "nc" not in _NC_CACHE:
        _NC_CACHE["nc"] = build_nc()
    nc = _NC_CACHE["nc"]

    shared = _prep_shared(W)
    in_maps = []
    for m in range(NC):
        per = _prep_core(x[m * BL : (m + 1) * BL])
        in_maps.append({**shared, **per})

    res = run_bass_kernel_spmd(nc, in_maps, list(range(NC)))
    out = np.empty((C, B, 1, 1, O), np.float32)
    for m in range(NC):
        o = res.results[m]["out"]                         # [C, O, BL]
        out[:, m * BL : (m + 1) * BL, 0, 0, :] = np.asarray(o).transpose(0, 2, 1)
    return out


if __name__ == "__main__":
    d = np.load("/root/problem/ref_data.npz")
    got = kernel(d["x"], d["W"])
    exp = d["expected"]
    err = np.abs(got - exp).max() / np.abs(exp).max()
    print("Relative error:", err)
